# revision 14
# baseline (speedup 1.0000x reference)
"""Trainium2 Bass kernel for nn_Attention_67336497266780.

Single-head attention, B=8 S=2048 E=1024 H=64, data-parallel over batch:
each of the 8 NeuronCores computes one batch element end to end.

Per-core algorithm (bf16 data path, software-pipelined PV tail):
  - q/k/v loaded via SWDGE cast-DMA (HBM f32 -> SBUF bf16) in 512-row
    blocks laid out so each partition holds 4 contiguous DRAM rows (128
    descriptors per block; the s-dim is consistently permuted and
    un-permuted at the output DMA).
  - PE transposes X in bf16 (1 cyc/col), 4 tiles per PSUM bank, one DVE
    drain [128,512] (as uint32) per bank-fill.
  - Projections with W stationary (bf16, N=512); bias via tensor_scalar;
    qt/kt duplicated on partition halves for row-group-packed scores.
  - scores^T per k-tile: 4 matmuls into two [128,1024] PSUM tiles with
    only 2 LDWs; exp on ACT (1024-wide) -> resident exp_all bf16.
  - PV^T: outT[65,512] per q-chunk = sum_m V'_m^T @ exp_m where
    V' = [V | 1] (ones column accumulates the softmax denominator);
    PE-transpose back, reciprocal normalize, DMA out.  In repeat mode
    the PV for iteration i runs at the START of iteration i+1
    interleaved with the q-phase DMA; a tail PV after the loop emits
    the final output.

Self-contained: hardcodes shapes; builds + compiles once per process and
caches the jitted PJRT executable for subsequent calls.
"""
import sys

try:
    import concourse  # noqa: F401  (resolves via PYTHONPATH when present)
except ImportError:
    sys.path.insert(0, "/opt/trn_rl_repo")

from contextlib import ExitStack

import numpy as np

import concourse.bass as bass
import concourse.mybir as mybir
import concourse.tile as tile
from concourse import bacc
from concourse.masks import make_identity

F32 = mybir.dt.float32
F32R = mybir.dt.float32r
BF16 = mybir.dt.bfloat16
U32 = mybir.dt.uint32

B = 8
P = 128
S = 2048
E = 1024
H = 64
EC = E // P          # 8 e-chunks
ST = S // P          # 16 s-tiles
NBLK = 4             # 4 blocks of 512 rows
BT = 4               # s-tiles per block
SBLK = BT * P        # 512
QCH = 512            # q-chunk width (one PSUM bank)

_CACHE = {}


def build(repeat=0, debug=False, unroll=0, xv_bufs=4, xt_bufs=3,
          all_hints=True, pv_head=True):
    nc = bacc.Bacc("TRN2", target_bir_lowering=False, debug=debug)

    xq_ext = nc.dram_tensor("query", [S, E], F32, kind="ExternalInput")
    xk_ext = nc.dram_tensor("key", [S, E], F32, kind="ExternalInput")
    xv_ext = nc.dram_tensor("value", [S, E], F32, kind="ExternalInput")
    wq_ext = nc.dram_tensor("Wq", [E, H], F32, kind="ExternalInput")
    wk_ext = nc.dram_tensor("Wk", [E, H], F32, kind="ExternalInput")
    wv_ext = nc.dram_tensor("Wv", [E, H], F32, kind="ExternalInput")
    bq_ext = nc.dram_tensor("bq", [H], F32, kind="ExternalInput")
    bk_ext = nc.dram_tensor("bk", [H], F32, kind="ExternalInput")
    bv_ext = nc.dram_tensor("bv", [H], F32, kind="ExternalInput")
    out_ext = nc.dram_tensor("out", [S, H], F32, kind="ExternalOutput")

    ctx = ExitStack()
    with tile.TileContext(nc) as tc, ctx:
        const = ctx.enter_context(tc.tile_pool(name="const", bufs=1))
        persist = ctx.enter_context(tc.tile_pool(name="persist", bufs=1))
        xv_pool = ctx.enter_context(tc.tile_pool(name="xv", bufs=xv_bufs))
        xt_pool = ctx.enter_context(tc.tile_pool(name="xt", bufs=xt_bufs))
        outt_pool = ctx.enter_context(tc.tile_pool(name="outt", bufs=2))
        ps_work = ctx.enter_context(tc.tile_pool(name="ps_work", bufs=2, space="PSUM"))
        ps_proj = ctx.enter_context(tc.tile_pool(name="ps_proj", bufs=2, space="PSUM"))
        ps_sc = ctx.enter_context(tc.tile_pool(name="ps_sc", bufs=2, space="PSUM"))

        # ---- constants (outside timing loop) ----
        ident = const.tile([P, P], BF16, name="ident")
        make_identity(nc, ident)
        ident_f = const.tile([P, P], F32, name="ident_f")
        make_identity(nc, ident_f)

        w_sb = {}
        b_sb = {}
        for name, wext, bext in (("q", wq_ext, bq_ext), ("k", wk_ext, bk_ext),
                                 ("v", wv_ext, bv_ext)):
            w_raw = const.tile([P, EC, H], F32, name=f"wraw{name}")
            nc.scalar.dma_start(w_raw[:], wext.rearrange("(o p) h -> p o h", p=P))
            w = const.tile([P, EC, H], BF16, name=f"w{name}")
            nc.scalar.copy(out=w[:], in_=w_raw[:])
            w_sb[name] = w
            b = const.tile([H, 1], F32, name=f"b{name}")
            nc.scalar.dma_start(b[:], bext[:].unsqueeze(1))
            b_sb[name] = b

        qt_sb = persist.tile([P, S], BF16, name="qt")       # dup halves
        kt_sb = persist.tile([P, S], BF16, name="kt")       # dup halves
        vt_sb = persist.tile([H, S], BF16, name="vt")
        vp_sb = persist.tile([P, ST, H + 2], BF16, name="vprime")  # col H=1s
        exp_all = persist.tile([P, ST, S], BF16, name="exp_all")
        out_sb = persist.tile([P, ST, H], F32, name="out_sb")
        rc_sb = persist.tile([P, ST], F32, name="rc")

        # init so iteration 0's PV-head reads defined data (ones column of
        # vp is load-bearing in every mode; exp_all only matters for the
        # PV-head, which exists only in repeat/unrolled mode)
        nc.vector.memset(vp_sb[:], 1.0)
        if repeat or unroll > 1:
            nc.gpsimd.memset(exp_all[:], 1.0)

        if repeat:
            if all_hints:
                hints = (mybir.EngineType.PE, mybir.EngineType.DVE,
                         mybir.EngineType.Activation, mybir.EngineType.SP,
                         mybir.EngineType.Pool)
            else:
                hints = (mybir.EngineType.PE, mybir.EngineType.DVE)
            loop_cm = tc.For_i(0, repeat, 1, hint_engines=hints)
        else:
            import contextlib
            loop_cm = contextlib.nullcontext()

        def load_block(xext, b):
            """SWDGE cast-DMA f32->bf16.  Partition p holds block rows
            4p..4p+3 (16KB contiguous DRAM per partition -> 128 descs)."""
            x_t = xv_pool.tile([P, BT, E], BF16, tag="xv")
            src = xext[b * SBLK:(b + 1) * SBLK, :].rearrange(
                "(p t) e -> p t e", p=P)
            nc.gpsimd.dma_start(x_t[:], src)
            return x_t

        def transpose_chunks(x_t, xt_t, chunks):
            """Transpose e-chunks `chunks` of x_t into xt_t."""
            for c in chunks:
                t_ps = ps_work.tile([P, SBLK], BF16, tag="ps")
                for t in range(BT):
                    nc.tensor.transpose(
                        t_ps[:, t * P:(t + 1) * P],
                        x_t[:, t, c * P:(c + 1) * P],
                        ident)
                nc.vector.tensor_copy(
                    out=xt_t[:, c].bitcast(U32),
                    in_=t_ps[:].bitcast(U32))

        def transpose_block(x_t):
            """x_t [128, BT, E] -> xt [128, EC, SBLK] (X^T for this block)."""
            xt_t = xt_pool.tile([P, EC, SBLK], BF16, tag="xt")
            for c in range(EC):
                t_ps = ps_work.tile([P, SBLK], BF16, tag="ps")
                for t in range(BT):
                    nc.tensor.transpose(
                        t_ps[:, t * P:(t + 1) * P],
                        x_t[:, t, c * P:(c + 1) * P],
                        ident)
                nc.vector.tensor_copy(
                    out=xt_t[:, c].bitcast(U32),
                    in_=t_ps[:].bitcast(U32))
            return xt_t

        def project_block(xt_t, b, tag):
            """xt block -> proj psum [H, SBLK]; drain to qt/kt (dup) or vt."""
            proj_ps = ps_proj.tile([H, SBLK], F32, tag="proj")
            w = w_sb[tag]
            for c in range(EC):
                nc.tensor.matmul(
                    proj_ps[:], lhsT=w[:, c], rhs=xt_t[:, c],
                    start=(c == 0), stop=(c == EC - 1))
            sl = slice(b * SBLK, (b + 1) * SBLK)
            bias = b_sb[tag]
            if tag == "v":
                nc.vector.tensor_scalar(
                    out=vt_sb[:, sl], in0=proj_ps[:], scalar1=bias,
                    scalar2=None, op0=mybir.AluOpType.add)
            else:
                dst = qt_sb if tag == "q" else kt_sb
                nc.vector.tensor_scalar(
                    out=dst[0:H, sl], in0=proj_ps[:], scalar1=bias,
                    scalar2=None, op0=mybir.AluOpType.add)
                nc.vector.tensor_scalar(
                    out=dst[H:2 * H, sl], in0=proj_ps[:], scalar1=bias,
                    scalar2=None, op0=mybir.AluOpType.add)

        def vprime_block(b):
            """vt block -> vp[:, m, 0:H] for the block's 4 m-tiles."""
            t_ps = ps_work.tile([P, BT * H], BF16, tag="ps")
            for t in range(BT):
                m = b * BT + t
                nc.tensor.transpose(
                    t_ps[:, t * H:(t + 1) * H],
                    vt_sb[:, m * P:(m + 1) * P],
                    ident[:H, :H])
            nc.vector.tensor_copy(
                out=vp_sb[:, b * BT:(b + 1) * BT, 0:H].bitcast(U32),
                in_=t_ps[:].rearrange("p (t h) -> p t h", t=BT).bitcast(U32))

        def scores_ktile(m):
            """scores^T for k-tile m: 4 matmuls, 2 LDWs (row-group packed),
            two [128,1024] PSUM tiles; exp 1024-wide."""
            mc = slice(m * P, (m + 1) * P)
            scA = ps_sc.tile([P, 2 * QCH], F32, tag="sc")
            scB = ps_sc.tile([P, 2 * QCH], F32, tag="sc")
            # row-group 0 (partitions 0:64): one LDW, two matmuls
            nc.tensor.matmul(scA[:, 0:QCH], lhsT=kt_sb[0:H, mc],
                             rhs=qt_sb[0:H, 0:QCH], start=True, stop=True)
            nc.tensor.matmul(scB[:, 0:QCH], lhsT=kt_sb[0:H, mc],
                             rhs=qt_sb[0:H, 2 * QCH:3 * QCH],
                             start=True, stop=True)
            # row-group 1 (partitions 64:128): one LDW, two matmuls
            nc.tensor.matmul(scA[:, QCH:2 * QCH], lhsT=kt_sb[H:2 * H, mc],
                             rhs=qt_sb[H:2 * H, QCH:2 * QCH],
                             start=True, stop=True)
            nc.tensor.matmul(scB[:, QCH:2 * QCH], lhsT=kt_sb[H:2 * H, mc],
                             rhs=qt_sb[H:2 * H, 3 * QCH:4 * QCH],
                             start=True, stop=True)
            nc.scalar.activation(
                exp_all[:, m, 0:2 * QCH], scA[:],
                mybir.ActivationFunctionType.Exp, scale=0.125)
            nc.scalar.activation(
                exp_all[:, m, 2 * QCH:4 * QCH], scB[:],
                mybir.ActivationFunctionType.Exp, scale=0.125)

        def pv_block(b):
            """outT[65, 512] for q-chunk b; transpose back, normalize, DMA."""
            pv = ps_sc.tile([H + 1, QCH], F32, tag="sc")
            for m in range(ST):
                nc.tensor.matmul(
                    pv[:], lhsT=vp_sb[:, m, 0:H + 1],
                    rhs=exp_all[:, m, b * QCH:(b + 1) * QCH],
                    start=(m == 0), stop=(m == ST - 1))
            outt = outt_pool.tile([H + 1, QCH], F32, tag="outt")
            nc.vector.tensor_copy(out=outt[:], in_=pv[:])
            for t in range(BT):
                qt_idx = b * BT + t
                o_ps = ps_work.tile([P, H + 1], F32, tag="ps")
                nc.tensor.transpose(
                    o_ps[:],
                    outt[:, t * P:(t + 1) * P],
                    ident_f[:H + 1, :H + 1])
                o_f = o_ps
                nc.vector.reciprocal(rc_sb[:, qt_idx:qt_idx + 1],
                                     o_f[:, H:H + 1])
                nc.vector.tensor_scalar(
                    out=out_sb[:, qt_idx], in0=o_f[:, 0:H],
                    scalar1=rc_sb[:, qt_idx:qt_idx + 1],
                    scalar2=None, op0=mybir.AluOpType.mult)
            nc.scalar.dma_start(
                out_ext[b * SBLK:(b + 1) * SBLK, :].rearrange(
                    "(p t) h -> p t h", p=P),
                out_sb[:, b * BT:(b + 1) * BT])

        def body(with_pv_head):
            # ---- phase Q (+ PV head of previous iteration) ----
            for b in range(NBLK):
                xb = load_block(xq_ext, b)
                if with_pv_head:
                    pv_block(b)
                xt_t = transpose_block(xb)
                project_block(xt_t, b, "q")
            # ---- phase K/V + scores + exp ----
            # The block's 4 score-tiles are interleaved with the v-side work
            # so the PE has transpose/proj matmuls to run while ACT exps the
            # previous score tile (avoids the scores<->exp PSUM ping-pong
            # head-of-line stall on the PE queue).
            for b in range(NBLK):
                xkb = load_block(xk_ext, b)
                xkt = transpose_block(xkb)
                project_block(xkt, b, "k")
                scores_ktile(b * BT + 0)
                xvb = load_block(xv_ext, b)
                xvt = xt_pool.tile([P, EC, SBLK], BF16, tag="xt")
                transpose_chunks(xvb, xvt, range(0, EC // 2))
                scores_ktile(b * BT + 1)
                transpose_chunks(xvb, xvt, range(EC // 2, EC))
                scores_ktile(b * BT + 2)
                project_block(xvt, b, "v")
                vprime_block(b)
                scores_ktile(b * BT + 3)

        if unroll:
            # python-unrolled steady-state emulation (for TimelineSim)
            for i in range(unroll):
                body(with_pv_head=(i > 0))
        else:
            with loop_cm:
                body(with_pv_head=bool(repeat) and pv_head)

        # ---- tail: final PV + output ----
        for b in range(NBLK):
            pv_block(b)

    nc.compile()
    return nc


def _get_runner():
    if "runner" in _CACHE:
        return _CACHE["runner"]

    import functools
    import traceback

    import jax
    from jax.experimental.shard_map import shard_map
    from jax.sharding import Mesh, PartitionSpec

    from concourse import bass2jax
    from concourse.bass2jax import _bass_exec_p, partition_id_tensor

    bass2jax.install_neuronx_cc_hook()
    import libneuronxla
    hook = libneuronxla.neuronx_cc
    if not getattr(hook, "_verbose_wrapped", False):
        @functools.wraps(hook)
        def wrapped(*a, **k):
            try:
                return hook(*a, **k)
            except BaseException:
                traceback.print_exc()
                sys.stderr.flush()
                raise
        wrapped._verbose_wrapped = True
        libneuronxla.neuronx_cc = wrapped

    nc = build()

    partition_name = nc.partition_id_tensor.name if nc.partition_id_tensor else None
    in_names, out_names, out_avals, zero_outs = [], [], [], []
    for alloc in nc.m.functions[0].allocations:
        if not isinstance(alloc, mybir.MemoryLocationSet):
            continue
        name = alloc.memorylocations[0].name
        if alloc.kind == "ExternalInput":
            if name != partition_name:
                in_names.append(name)
        elif alloc.kind == "ExternalOutput":
            out_names.append(name)
            shape = tuple(alloc.tensor_shape)
            dtype = mybir.dt.np(alloc.dtype)
            out_avals.append(jax.core.ShapedArray(shape, dtype))
            zero_outs.append(np.zeros(shape, dtype))
    n_params = len(in_names)
    n_outs = len(out_avals)
    all_in_names = list(in_names) + out_names
    if partition_name is not None:
        all_in_names.append(partition_name)
    donate = tuple(range(n_params, n_params + n_outs))

    def _body(*args):
        operands = list(args)
        if partition_name is not None:
            operands.append(partition_id_tensor())
        outs = _bass_exec_p.bind(
            *operands,
            out_avals=tuple(out_avals),
            in_names=tuple(all_in_names),
            out_names=tuple(out_names),
            lowering_input_output_aliases=(),
            sim_require_finite=False,
            sim_require_nnan=False,
            nc=nc,
        )
        return tuple(outs)

    devices = jax.devices()[:B]
    mesh = Mesh(np.asarray(devices), ("core",))
    in_specs = (PartitionSpec("core"),) * (n_params + n_outs)
    out_specs = (PartitionSpec("core"),) * len(out_names)
    sharded = jax.jit(
        shard_map(_body, mesh=mesh, in_specs=in_specs,
                  out_specs=out_specs, check_rep=False),
        donate_argnums=donate, keep_unused=True)

    runner = {
        "sharded": sharded, "in_names": in_names, "out_names": out_names,
        "out_avals": out_avals, "zero_outs": zero_outs,
    }
    _CACHE["runner"] = runner
    return runner


def kernel(**inputs):
    r = _get_runner()
    per_core = {"query", "key", "value"}

    concat_in = []
    for name in r["in_names"]:
        arr = np.ascontiguousarray(np.asarray(inputs[name], dtype=np.float32))
        if name in per_core:
            concat_in.append(arr.reshape(B * S, E))
        else:
            concat_in.append(np.concatenate([arr] * B, axis=0))
    concat_zeros = [
        np.zeros((B * z.shape[0], *z.shape[1:]), z.dtype) for z in r["zero_outs"]
    ]
    out_arrs = r["sharded"](*concat_in, *concat_zeros)
    (aval,) = r["out_avals"]
    out = np.asarray(out_arrs[0]).reshape(B, *aval.shape)
    return out.astype(np.float32, copy=False)


if __name__ == "__main__":
    rng = np.random.default_rng(0)
    fake = {
        "query": rng.standard_normal((B, S, E), dtype=np.float32),
        "key": rng.standard_normal((B, S, E), dtype=np.float32),
        "value": rng.standard_normal((B, S, E), dtype=np.float32),
        "Wq": rng.standard_normal((E, H), dtype=np.float32) / 32,
        "bq": np.zeros(H, np.float32),
        "Wk": rng.standard_normal((E, H), dtype=np.float32) / 32,
        "bk": np.zeros(H, np.float32),
        "Wv": rng.standard_normal((E, H), dtype=np.float32) / 32,
        "bv": np.zeros(H, np.float32),
    }
    out = kernel(**fake)
    print("kernel out:", out.shape, out.dtype, float(out[0, 0, 0]))


# revision 17
# speedup vs baseline: 1.4404x; 1.4404x over previous
"""Trainium2 Bass kernel for nn_Attention_67336497266780.

Single-head attention, B=8 S=2048 E=1024 H=64, data-parallel over batch:
each of the 8 NeuronCores computes one batch element end to end.

Per-core algorithm (bf16 data path, software-pipelined PV tail):
  - q/k/v loaded via SWDGE cast-DMA (HBM f32 -> SBUF bf16) in 512-row
    blocks laid out so each partition holds 4 contiguous DRAM rows (128
    descriptors per block; the s-dim is consistently permuted and
    un-permuted at the output DMA).
  - PE transposes X in bf16 (1 cyc/col), 4 tiles per PSUM bank, one DVE
    drain [128,512] (as uint32) per bank-fill.
  - Projections with W stationary (bf16, N=512); bias via tensor_scalar;
    qt/kt duplicated on partition halves for row-group-packed scores.
  - scores^T per k-tile: 4 matmuls into two [128,1024] PSUM tiles with
    only 2 LDWs; exp on ACT (1024-wide) -> resident exp_all bf16.
  - PV^T: outT[65,512] per q-chunk = sum_m V'_m^T @ exp_m where
    V' = [V | 1] (ones column accumulates the softmax denominator);
    PE-transpose back, reciprocal normalize, DMA out.  In repeat mode
    the PV for iteration i runs at the START of iteration i+1
    interleaved with the q-phase DMA; a tail PV after the loop emits
    the final output.

Self-contained: hardcodes shapes; builds + compiles once per process and
caches the jitted PJRT executable for subsequent calls.
"""
import sys

try:
    import concourse  # noqa: F401  (resolves via PYTHONPATH when present)
except ImportError:
    sys.path.insert(0, "/opt/trn_rl_repo")

from contextlib import ExitStack

import numpy as np

import concourse.bass as bass
import concourse.mybir as mybir
import concourse.tile as tile
from concourse import bacc
from concourse.masks import make_identity

F32 = mybir.dt.float32
F32R = mybir.dt.float32r
BF16 = mybir.dt.bfloat16
U32 = mybir.dt.uint32

B = 8
P = 128
S = 2048
E = 1024
H = 64
EC = E // P          # 8 e-chunks
ST = S // P          # 16 s-tiles
NBLK = 4             # 4 blocks of 512 rows
BT = 4               # s-tiles per block
SBLK = BT * P        # 512
QCH = 512            # q-chunk width (one PSUM bank)

_CACHE = {}


def build(repeat=0, debug=False, unroll=0, xv_bufs=4, xt_bufs=3,
          all_hints=True, pv_head=True):
    nc = bacc.Bacc("TRN2", target_bir_lowering=False, debug=debug)

    xq_ext = nc.dram_tensor("query", [S, E], F32, kind="ExternalInput")
    xk_ext = nc.dram_tensor("key", [S, E], F32, kind="ExternalInput")
    xv_ext = nc.dram_tensor("value", [S, E], F32, kind="ExternalInput")
    wq_ext = nc.dram_tensor("Wq", [E, H], F32, kind="ExternalInput")
    wk_ext = nc.dram_tensor("Wk", [E, H], F32, kind="ExternalInput")
    wv_ext = nc.dram_tensor("Wv", [E, H], F32, kind="ExternalInput")
    bq_ext = nc.dram_tensor("bq", [H], F32, kind="ExternalInput")
    bk_ext = nc.dram_tensor("bk", [H], F32, kind="ExternalInput")
    bv_ext = nc.dram_tensor("bv", [H], F32, kind="ExternalInput")
    out_ext = nc.dram_tensor("out", [S, H], F32, kind="ExternalOutput")

    ctx = ExitStack()
    with tile.TileContext(nc) as tc, ctx:
        const = ctx.enter_context(tc.tile_pool(name="const", bufs=1))
        persist = ctx.enter_context(tc.tile_pool(name="persist", bufs=1))
        xv_pool = ctx.enter_context(tc.tile_pool(name="xv", bufs=xv_bufs))
        xt_pool = ctx.enter_context(tc.tile_pool(name="xt", bufs=xt_bufs))
        outt_pool = ctx.enter_context(tc.tile_pool(name="outt", bufs=2))
        ps_work = ctx.enter_context(tc.tile_pool(name="ps_work", bufs=2, space="PSUM"))
        ps_proj = ctx.enter_context(tc.tile_pool(name="ps_proj", bufs=2, space="PSUM"))
        ps_sc = ctx.enter_context(tc.tile_pool(name="ps_sc", bufs=2, space="PSUM"))

        # ---- constants (outside timing loop) ----
        ident = const.tile([P, P], BF16, name="ident")
        make_identity(nc, ident)
        ident_f = const.tile([P, P], F32, name="ident_f")
        make_identity(nc, ident_f)

        w_sb = {}
        b_sb = {}
        for name, wext, bext in (("q", wq_ext, bq_ext), ("k", wk_ext, bk_ext),
                                 ("v", wv_ext, bv_ext)):
            w_raw = const.tile([P, EC, H], F32, name=f"wraw{name}")
            nc.scalar.dma_start(w_raw[:], wext.rearrange("(o p) h -> p o h", p=P))
            w = const.tile([P, EC, H], BF16, name=f"w{name}")
            nc.scalar.copy(out=w[:], in_=w_raw[:])
            w_sb[name] = w
            b = const.tile([H, 1], F32, name=f"b{name}")
            nc.scalar.dma_start(b[:], bext[:].unsqueeze(1))
            b_sb[name] = b

        qt_sb = persist.tile([P, S], BF16, name="qt")       # dup halves
        kt_sb = persist.tile([P, S], BF16, name="kt")       # dup halves
        vt_sb = persist.tile([H, S], BF16, name="vt")
        vp_sb = persist.tile([P, ST, H + 2], BF16, name="vprime")  # col H=1s
        exp_all = persist.tile([P, ST, S], BF16, name="exp_all")
        out_sb = persist.tile([P, ST, H], F32, name="out_sb")
        rc_sb = persist.tile([P, ST], F32, name="rc")

        # init so iteration 0's PV-head reads defined data (ones column of
        # vp is load-bearing in every mode; exp_all only matters for the
        # PV-head, which exists only in repeat/unrolled mode)
        nc.vector.memset(vp_sb[:], 1.0)
        if repeat or unroll > 1:
            nc.gpsimd.memset(exp_all[:], 1.0)

        if repeat:
            if all_hints:
                hints = (mybir.EngineType.PE, mybir.EngineType.DVE,
                         mybir.EngineType.Activation, mybir.EngineType.SP,
                         mybir.EngineType.Pool)
            else:
                hints = (mybir.EngineType.PE, mybir.EngineType.DVE)
            loop_cm = tc.For_i(0, repeat, 1, hint_engines=hints)
        else:
            import contextlib
            loop_cm = contextlib.nullcontext()

        def load_block(xext, b):
            """SWDGE cast-DMA f32->bf16.  Partition p holds block rows
            4p..4p+3 (16KB contiguous DRAM per partition -> 128 descs)."""
            x_t = xv_pool.tile([P, BT, E], BF16, tag="xv")
            src = xext[b * SBLK:(b + 1) * SBLK, :].rearrange(
                "(p t) e -> p t e", p=P)
            nc.gpsimd.dma_start(x_t[:], src)
            return x_t

        def transpose_chunks(x_t, xt_t, chunks):
            """Transpose e-chunks `chunks` of x_t into xt_t."""
            for c in chunks:
                t_ps = ps_work.tile([P, SBLK], BF16, tag="ps")
                for t in range(BT):
                    nc.tensor.transpose(
                        t_ps[:, t * P:(t + 1) * P],
                        x_t[:, t, c * P:(c + 1) * P],
                        ident)
                nc.vector.tensor_copy(
                    out=xt_t[:, c].bitcast(U32),
                    in_=t_ps[:].bitcast(U32))

        def transpose_block(x_t):
            """x_t [128, BT, E] -> xt [128, EC, SBLK] (X^T for this block)."""
            xt_t = xt_pool.tile([P, EC, SBLK], BF16, tag="xt")
            for c in range(EC):
                t_ps = ps_work.tile([P, SBLK], BF16, tag="ps")
                for t in range(BT):
                    nc.tensor.transpose(
                        t_ps[:, t * P:(t + 1) * P],
                        x_t[:, t, c * P:(c + 1) * P],
                        ident)
                nc.vector.tensor_copy(
                    out=xt_t[:, c].bitcast(U32),
                    in_=t_ps[:].bitcast(U32))
            return xt_t

        def project_block(xt_t, b, tag):
            """xt block -> proj psum [H, SBLK]; drain to qt/kt (dup) or vt."""
            proj_ps = ps_proj.tile([H, SBLK], F32, tag="proj")
            w = w_sb[tag]
            for c in range(EC):
                nc.tensor.matmul(
                    proj_ps[:], lhsT=w[:, c], rhs=xt_t[:, c],
                    start=(c == 0), stop=(c == EC - 1))
            sl = slice(b * SBLK, (b + 1) * SBLK)
            bias = b_sb[tag]
            if tag == "v":
                nc.vector.tensor_scalar(
                    out=vt_sb[:, sl], in0=proj_ps[:], scalar1=bias,
                    scalar2=None, op0=mybir.AluOpType.add)
            else:
                dst = qt_sb if tag == "q" else kt_sb
                nc.vector.tensor_scalar(
                    out=dst[0:H, sl], in0=proj_ps[:], scalar1=bias,
                    scalar2=None, op0=mybir.AluOpType.add)
                nc.vector.tensor_scalar(
                    out=dst[H:2 * H, sl], in0=proj_ps[:], scalar1=bias,
                    scalar2=None, op0=mybir.AluOpType.add)

        def vprime_block(b):
            """vt block -> vp[:, m, 0:H] for the block's 4 m-tiles."""
            t_ps = ps_work.tile([P, BT * H], BF16, tag="ps")
            for t in range(BT):
                m = b * BT + t
                nc.tensor.transpose(
                    t_ps[:, t * H:(t + 1) * H],
                    vt_sb[:, m * P:(m + 1) * P],
                    ident[:H, :H])
            nc.vector.tensor_copy(
                out=vp_sb[:, b * BT:(b + 1) * BT, 0:H].bitcast(U32),
                in_=t_ps[:].rearrange("p (t h) -> p t h", t=BT).bitcast(U32))

        def scores_ktile(m):
            """scores^T for k-tile m: 4 matmuls, 2 LDWs (row-group packed),
            two [128,1024] PSUM tiles; exp 1024-wide."""
            mc = slice(m * P, (m + 1) * P)
            scA = ps_sc.tile([P, 2 * QCH], F32, tag="sc")
            scB = ps_sc.tile([P, 2 * QCH], F32, tag="sc")
            # row-group 0 (partitions 0:64): one LDW, two matmuls
            nc.tensor.matmul(scA[:, 0:QCH], lhsT=kt_sb[0:H, mc],
                             rhs=qt_sb[0:H, 0:QCH], start=True, stop=True)
            nc.tensor.matmul(scB[:, 0:QCH], lhsT=kt_sb[0:H, mc],
                             rhs=qt_sb[0:H, 2 * QCH:3 * QCH],
                             start=True, stop=True)
            # row-group 1 (partitions 64:128): one LDW, two matmuls
            nc.tensor.matmul(scA[:, QCH:2 * QCH], lhsT=kt_sb[H:2 * H, mc],
                             rhs=qt_sb[H:2 * H, QCH:2 * QCH],
                             start=True, stop=True)
            nc.tensor.matmul(scB[:, QCH:2 * QCH], lhsT=kt_sb[H:2 * H, mc],
                             rhs=qt_sb[H:2 * H, 3 * QCH:4 * QCH],
                             start=True, stop=True)
            nc.scalar.activation(
                exp_all[:, m, 0:2 * QCH], scA[:],
                mybir.ActivationFunctionType.Exp, scale=0.125)
            nc.scalar.activation(
                exp_all[:, m, 2 * QCH:4 * QCH], scB[:],
                mybir.ActivationFunctionType.Exp, scale=0.125)

        def pv_block(b):
            """outT[65, 512] for q-chunk b; transpose back, normalize, DMA."""
            pv = ps_sc.tile([H + 1, QCH], F32, tag="sc")
            for m in range(ST):
                nc.tensor.matmul(
                    pv[:], lhsT=vp_sb[:, m, 0:H + 1],
                    rhs=exp_all[:, m, b * QCH:(b + 1) * QCH],
                    start=(m == 0), stop=(m == ST - 1))
            outt = outt_pool.tile([H + 1, QCH], F32, tag="outt")
            nc.vector.tensor_copy(out=outt[:], in_=pv[:])
            for t in range(BT):
                qt_idx = b * BT + t
                o_ps = ps_work.tile([P, H + 1], F32, tag="ps")
                nc.tensor.transpose(
                    o_ps[:],
                    outt[:, t * P:(t + 1) * P],
                    ident_f[:H + 1, :H + 1])
                o_f = o_ps
                nc.vector.reciprocal(rc_sb[:, qt_idx:qt_idx + 1],
                                     o_f[:, H:H + 1])
                nc.vector.tensor_scalar(
                    out=out_sb[:, qt_idx], in0=o_f[:, 0:H],
                    scalar1=rc_sb[:, qt_idx:qt_idx + 1],
                    scalar2=None, op0=mybir.AluOpType.mult)
            nc.scalar.dma_start(
                out_ext[b * SBLK:(b + 1) * SBLK, :].rearrange(
                    "(p t) h -> p t h", p=P),
                out_sb[:, b * BT:(b + 1) * BT])

        def body(with_pv_head):
            # ---- phase Q (+ PV head of previous iteration) ----
            for b in range(NBLK):
                xb = load_block(xq_ext, b)
                if with_pv_head:
                    pv_block(b)
                xt_t = transpose_block(xb)
                project_block(xt_t, b, "q")
            # ---- phase K/V + scores + exp ----
            # The block's 4 score-tiles are interleaved with the v-side work
            # so the PE has transpose/proj matmuls to run while ACT exps the
            # previous score tile (avoids the scores<->exp PSUM ping-pong
            # head-of-line stall on the PE queue).
            for b in range(NBLK):
                xkb = load_block(xk_ext, b)
                xkt = transpose_block(xkb)
                project_block(xkt, b, "k")
                scores_ktile(b * BT + 0)
                xvb = load_block(xv_ext, b)
                xvt = xt_pool.tile([P, EC, SBLK], BF16, tag="xt")
                transpose_chunks(xvb, xvt, range(0, EC // 2))
                scores_ktile(b * BT + 1)
                transpose_chunks(xvb, xvt, range(EC // 2, EC))
                scores_ktile(b * BT + 2)
                project_block(xvt, b, "v")
                vprime_block(b)
                scores_ktile(b * BT + 3)

        if unroll:
            # python-unrolled steady-state emulation (for TimelineSim)
            for i in range(unroll):
                body(with_pv_head=(i > 0))
        else:
            with loop_cm:
                body(with_pv_head=bool(repeat) and pv_head)

        # ---- tail: final PV + output ----
        for b in range(NBLK):
            pv_block(b)

    nc.compile()
    return nc


def _get_runner():
    if "runner" in _CACHE:
        return _CACHE["runner"]

    import functools
    import traceback

    import jax
    from jax.experimental.shard_map import shard_map
    from jax.sharding import Mesh, PartitionSpec

    from concourse import bass2jax
    from concourse.bass2jax import _bass_exec_p, partition_id_tensor

    bass2jax.install_neuronx_cc_hook()
    import libneuronxla
    hook = libneuronxla.neuronx_cc
    if not getattr(hook, "_verbose_wrapped", False):
        @functools.wraps(hook)
        def wrapped(*a, **k):
            try:
                return hook(*a, **k)
            except BaseException:
                traceback.print_exc()
                sys.stderr.flush()
                raise
        wrapped._verbose_wrapped = True
        libneuronxla.neuronx_cc = wrapped

    nc = build()

    partition_name = nc.partition_id_tensor.name if nc.partition_id_tensor else None
    in_names, out_names, out_avals, zero_outs = [], [], [], []
    for alloc in nc.m.functions[0].allocations:
        if not isinstance(alloc, mybir.MemoryLocationSet):
            continue
        name = alloc.memorylocations[0].name
        if alloc.kind == "ExternalInput":
            if name != partition_name:
                in_names.append(name)
        elif alloc.kind == "ExternalOutput":
            out_names.append(name)
            shape = tuple(alloc.tensor_shape)
            dtype = mybir.dt.np(alloc.dtype)
            out_avals.append(jax.core.ShapedArray(shape, dtype))
            zero_outs.append(np.zeros(shape, dtype))
    n_params = len(in_names)
    n_outs = len(out_avals)
    all_in_names = list(in_names) + out_names
    if partition_name is not None:
        all_in_names.append(partition_name)
    donate = tuple(range(n_params, n_params + n_outs))

    def _body(*args):
        operands = list(args)
        if partition_name is not None:
            operands.append(partition_id_tensor())
        outs = _bass_exec_p.bind(
            *operands,
            out_avals=tuple(out_avals),
            in_names=tuple(all_in_names),
            out_names=tuple(out_names),
            lowering_input_output_aliases=(),
            sim_require_finite=False,
            sim_require_nnan=False,
            nc=nc,
        )
        return tuple(outs)

    devices = jax.devices()[:B]
    mesh = Mesh(np.asarray(devices), ("core",))
    in_specs = (PartitionSpec("core"),) * (n_params + n_outs)
    out_specs = (PartitionSpec("core"),) * len(out_names)
    sharded = jax.jit(
        shard_map(_body, mesh=mesh, in_specs=in_specs,
                  out_specs=out_specs, check_rep=False),
        donate_argnums=donate, keep_unused=True)

    runner = {
        "sharded": sharded, "in_names": in_names, "out_names": out_names,
        "out_avals": out_avals, "zero_outs": zero_outs,
    }
    _CACHE["runner"] = runner
    return runner


def kernel(**inputs):
    r = _get_runner()
    per_core = {"query", "key", "value"}

    concat_in = []
    for name in r["in_names"]:
        arr = np.ascontiguousarray(np.asarray(inputs[name], dtype=np.float32))
        if name in per_core:
            concat_in.append(arr.reshape(B * S, E))
        else:
            concat_in.append(np.concatenate([arr] * B, axis=0))
    concat_zeros = [
        np.zeros((B * z.shape[0], *z.shape[1:]), z.dtype) for z in r["zero_outs"]
    ]
    out_arrs = r["sharded"](*concat_in, *concat_zeros)
    (aval,) = r["out_avals"]
    out = np.asarray(out_arrs[0]).reshape(B, *aval.shape)
    return out.astype(np.float32, copy=False)


if __name__ == "__main__":
    rng = np.random.default_rng(0)
    fake = {
        "query": rng.standard_normal((B, S, E), dtype=np.float32),
        "key": rng.standard_normal((B, S, E), dtype=np.float32),
        "value": rng.standard_normal((B, S, E), dtype=np.float32),
        "Wq": rng.standard_normal((E, H), dtype=np.float32) / 32,
        "bq": np.zeros(H, np.float32),
        "Wk": rng.standard_normal((E, H), dtype=np.float32) / 32,
        "bk": np.zeros(H, np.float32),
        "Wv": rng.standard_normal((E, H), dtype=np.float32) / 32,
        "bv": np.zeros(H, np.float32),
    }
    out = kernel(**fake)
    print("kernel out:", out.shape, out.dtype, float(out[0, 0, 0]))
